# revision 17
# baseline (speedup 1.0000x reference)
"""BinASPP Trainium2 kernel (Bass/Tile), SPMD over 8 NeuronCores.

Strategy (v4)
-------------
Data-parallel over batch: N=8 images -> 1 image per core.  binarize()
forward == sign(), so every conv is a matmul over {-1,+1} values: exact in
fp8e4 with fp32 PSUM accumulation.  A dilated 3x3 conv is 9 shifted 1x1
convs (taps) over one zero-padded sign image (pad 12, 88x88) resident in
SBUF; each DoubleRow matmul contracts all K=256 input channels and streams
a 4D access pattern [k2, 8 rows, 64 cols] -- only useful columns, no pad
streaming.  8-row tiles fill one 2KB PSUM bank exactly; the matmul stream
is gapless at the fp8 peak (~0.43 ns/col, ~133 us).

Sync-BN uses five AllGathers of per-branch (mean, var): {pool,1x1,r1}
{r12 mc0} {r4} {r8} {r12 mc1}.  Section order r1/1x1, r12-mc0, r4, r8,
r12-mc1 spaces the collectives so the CC cores are free when the last
half-gather's payload lands (a collective costs ~2us start + ~15-22us, and
consecutive ccs serialize), and r12-mc0's output ships inside the final
gather's window.  Only {r12 mc1}'s gather + one merge + three fp16 passes
remain in the tail.

Engine budget: ACT does x sign passes + [128,512] PSUM->SBUF fp16 drains
(y fp16 exact: even integers <= 2304).  DVE does per-tile bn_stats (512
chunks, right after each drain), the clip/scale/accumulate applies
(3 passes: tensor_scalar clip, tensor_scalar scale, tensor_tensor add --
scalar_tensor_tensor runs at 1 elem/cycle, slower than ts+tt), and the
late merges.  Applies are emitted as 2048-col chunks interleaved between
matmul units so the in-order DVE queue never blocks a section's stats
chain (which gates the next collective).  Pool does pad memsets, the
early-group moment merges (pinned between collective triggers with
scheduling-only deps), and the collective triggers.  BN offsets d_j (+ the
pool branch's a*y_pool + d) fold into a per-channel s0 applied in the tail
passes; output is fp16, widened on host.
"""

import numpy as np
import ml_dtypes
from contextlib import ExitStack

import concourse.bass as bass
import concourse.bacc as bacc
import concourse.mybir as mybir
import concourse.tile as tile
from concourse.tile import add_dep_helper
from concourse.bass_utils import run_bass_kernel_spmd

AF = mybir.ActivationFunctionType
ALU = mybir.AluOpType
AX = mybir.AxisListType
F32 = mybir.dt.float32
F16 = mybir.dt.float16
FP8 = mybir.dt.float8e4
DR = mybir.MatmulPerfMode.DoubleRow

P = 128
CIN = 256
COUT = 256
H = W = 64
HW = H * W
PAD = 12
PW = H + 2 * PAD              # 88
ILEN = PW * PW                # 7744 (multiple of 16 -> DR k-stride rule)
RATES = (1, 4, 8, 12)
NT = 2 + 9 * len(RATES)       # 38 tap matrices
NBLK1 = 22                    # lhsT blocks for pool/1x1/r1 (shipped first)
EPS = 1e-5
N_CORES = 8
TROWS = 8                     # rows per PSUM tile (8*64 = 512 f32 = 1 bank)
NTILE = H // TROWS            # 8 tiles per branch-mc
# branch ids: 0=pool, 1=1x1, 2=r1, 3=r4, 4=r8, 5=r12
BR = {1: (1, None), 2: (2, 1), 3: (11, 4), 4: (20, 8), 5: (29, 12)}
# stats slots (j, mc) grouped per AllGather, in gather order; G0's slots
# are mc-outer so its d-sum reduces map onto s0's [mc] layout.
GROUPS = [[(0, 0), (1, 0), (2, 0), (0, 1), (1, 1), (2, 1)],
          [(5, 0)], [(3, 0), (3, 1)], [(4, 0), (4, 1)], [(5, 1)]]
GOFF = [0, 6, 7, 9, 11]       # slot offset of each group in the gb pack
SLOT = {}
for _g, _members in enumerate(GROUPS):
    for _si, _jm in enumerate(_members):
        SLOT[_jm] = (_g, _si)
# x row-blocks: small first block so the first matmul starts early
XBLKS = [(0, 12), (12, 12), (24, 20), (44, 20)]


def build(n_cores: int = N_CORES):
    nc = bacc.Bacc(
        "TRN2",
        target_bir_lowering=False,
        debug=False,
        enable_asserts=False,
        num_devices=n_cores,
    )
    xs = nc.dram_tensor("xs", [CIN, H, W], F32, kind="ExternalInput")
    wt = nc.dram_tensor("wt", [P, NT * 2, 2, P], FP8, kind="ExternalInput")
    gb = nc.dram_tensor("gb", [P, 12, 2], F32, kind="ExternalInput")
    out = nc.dram_tensor("out", [COUT, H, W], F16, kind="ExternalOutput")

    with tile.TileContext(nc) as tc, ExitStack() as ctx:
        const = ctx.enter_context(tc.tile_pool(name="const", bufs=1))
        xload = ctx.enter_context(tc.tile_pool(name="xload", bufs=2))
        ppool = ctx.enter_context(
            tc.tile_pool(name="ppool", bufs=6, space=bass.MemorySpace.PSUM))
        psmall = ctx.enter_context(
            tc.tile_pool(name="psmall", bufs=2, space=bass.MemorySpace.PSUM))
        ybuf = ctx.enter_context(tc.tile_pool(name="ybuf", bufs=1))
        stat = ctx.enter_context(tc.tile_pool(name="stat", bufs=1))
        tmp = ctx.enter_context(tc.tile_pool(name="tmp", bufs=4))
        sbout = ctx.enter_context(tc.tile_pool(name="sbout", bufs=4))
        dram = ctx.enter_context(
            tc.tile_pool(name="dram", bufs=1, space=bass.MemorySpace.DRAM))

        lhsT = const.tile([P, NT * 2, 2, P], FP8, tag="lhsT")
        gb_sb = const.tile([P, 12, 2], F32, tag="gb")
        nc.scalar.dma_start(gb_sb[:], gb.ap())

        def wdr(blk):
            return lhsT[:, blk]          # [P, 2, P] fp8, k-interleaved

        # ---- padded k-interleaved sign image.  Top pad on DVE (gates r1
        # tile 0), seams/bottom on Pool.  DMA+sign+reduce interleaved per
        # block so Tile's coalesced DMA-queue semaphores never make a sign
        # wait on later blocks' loads.
        sxp = const.tile([P, 2, ILEN], FP8, tag="sxp")
        img4 = sxp[:].rearrange("p k (r c) -> p k r c", c=PW)
        interiors = [img4[:, kc, PAD:PAD + H, PAD:PAD + W] for kc in range(2)]
        spool = const.tile([P, 2, 16], FP8, tag="spool")  # 16-wide: DR k rule
        xs10 = xload.tile([P, 2, len(XBLKS)], F32, tag="xs10", name="xs10")
        seam0 = (PAD + 1) * PW - PAD
        seams = sxp[:, :, seam0:seam0 + (H - 1) * PW].rearrange(
            "p k (r c) -> p k r c", c=PW)[:, :, :, 0:2 * PAD]

        nc.vector.memset(sxp[:, :, 0:PAD * PW + PAD], 0.0)
        for bi, (r0, nr) in enumerate(XBLKS):
            xsb0 = xload.tile([P, nr, W], F32, tag=f"xsb{nr}_0")
            xsb1 = xload.tile([P, nr, W], F32, tag=f"xsb{nr}_1")
            nc.sync.dma_start(xsb0[:], xs.ap()[0:P, r0:r0 + nr])
            eng1 = nc.scalar if bi == 0 else nc.gpsimd
            eng1.dma_start(xsb1[:], xs.ap()[P:2 * P, r0:r0 + nr])
            nc.scalar.activation(interiors[0][:, r0:r0 + nr], xsb0[:],
                                 AF.Sign)
            nc.scalar.activation(interiors[1][:, r0:r0 + nr], xsb1[:],
                                 AF.Sign)
            nc.vector.reduce_sum(xs10[:, 0, bi:bi + 1], xsb0[:], axis=AX.XY)
            nc.vector.reduce_sum(xs10[:, 1, bi:bi + 1], xsb1[:], axis=AX.XY)
            if bi == 0:
                nc.sync.dma_start(lhsT[:, 0:NBLK1], wt.ap()[:, 0:NBLK1])
                nc.gpsimd.memset(seams, 0.0)
            if bi == 1:
                nc.gpsimd.memset(sxp[:, :, (PAD + H) * PW - PAD:ILEN], 0.0)
        xsum = xload.tile([P, 2], F32, tag="xsum", name="xsum")
        nc.vector.reduce_sum(xsum[:], xs10[:], axis=AX.X)
        nc.scalar.activation(spool[:, :, 0], xsum[:], AF.Sign)
        nc.sync.dma_start(lhsT[:, NBLK1:], wt.ap()[:, NBLK1:])

        # per-group stats [P, S, 2] = (mean, var) per slot
        stats_g = [stat.tile([P, len(m), 2], F32, tag=f"stats_g{g}",
                             name=f"stats_g{g}") for g, m in enumerate(GROUPS)]
        bn6 = {j: stat.tile([P, 2, NTILE, 6], F32, tag=f"bn6_{j}",
                            name=f"bn6_{j}") for j in BR}
        s0 = stat.tile([P, 2], F32, tag="s0", name="s0")
        nc.vector.memset(s0[:], 0.0)

        y16_all = {j: {mc: ybuf.tile([P, HW], F16, tag=f"y{j}_{mc}",
                                     name=f"y{j}_{mc}") for mc in range(2)}
                   for j in BR}
        q = {mc: y16_all[2][mc] for mc in range(2)}  # r1's tiles hold the sum
        ypool = {}
        coefs = {}
        gathers = {}
        cc_insts = {}

        def emit_unit(j, mc, t):
            tap0, r = BR[j]
            taps = ([(tap0, 1, 1)] if r is None else
                    [(tap0 + 3 * ky + kx, ky, kx)
                     for ky in range(3) for kx in range(3)])
            h0 = TROWS * t
            acc = ppool.tile([P, TROWS * W], F32, tag="acc")
            for i_mm, (tap, ky, kx) in enumerate(taps):
                rr = 0 if r is None else r
                rbase = PAD + h0 + rr * (ky - 1)
                cbase = PAD + rr * (kx - 1)
                rhs = img4[:, :, rbase:rbase + TROWS, cbase:cbase + W]
                nc.tensor.matmul(acc[:], wdr(tap * 2 + mc), rhs,
                                 start=(i_mm == 0),
                                 stop=(i_mm == len(taps) - 1),
                                 perf_mode=DR)
            ysl = y16_all[j][mc][:, h0 * W:(h0 + TROWS) * W]
            nc.scalar.activation(ysl, acc[:], AF.Copy)
            nc.vector.bn_stats(bn6[j][:, mc, t], ysl)
            if t == NTILE - 1:
                g, si = SLOT[(j, mc)]
                nc.vector.bn_aggr(stats_g[g][:, si], bn6[j][:, mc])

        def emit_pool_branch():
            for mc in range(2):
                yp = psmall.tile([P, 1], F32, tag="yp")
                nc.tensor.matmul(yp[:], wdr(0 * 2 + mc), spool[:, :, 0:1],
                                 start=True, stop=True, perf_mode=DR)
                ys = stat.tile([P, 1], F32, tag=f"ypool{mc}",
                               name=f"ypool{mc}")
                nc.scalar.activation(ys[:], yp[:], AF.Copy)
                ypool[mc] = ys
                g, si = SLOT[(0, mc)]
                nc.vector.tensor_copy(stats_g[g][:, si, 0:1], ys[:])
                nc.vector.memset(stats_g[g][:, si, 1:2], 0.0)

        def issue_gather(g):
            S = len(GROUPS[g])
            st_in = dram.tile([P, S * 2], F32, tag=f"st_in{g}")
            st_out = dram.tile([8, P, S * 2], F32, tag=f"st_out{g}",
                               addr_space="Shared" if n_cores > 4 else "Local")
            nc.sync.dma_start(st_in[:], stats_g[g][:])
            cc = nc.gpsimd.collective_compute(
                "AllGather", ALU.bypass,
                replica_groups=[list(range(n_cores))],
                ins=[st_in[:].opt()], outs=[st_out[:].opt()],
            )
            cc_insts[g] = cc
            gathers[g] = st_out

        def fetch_gather(g, eng=None):
            """Readback DMA on the ACT queue (sync carries st_ins/outs)."""
            S = len(GROUPS[g])
            gath = stat.tile([P, 8, S * 2], F32, tag=f"gath{g}",
                             name=f"gath{g}")
            (eng or nc.scalar).dma_start(
                gath[:], gathers[g][:].rearrange("c p f -> p c f"))
            gathers[g] = gath

        def emit_merge(g, on_pool):
            """Cross-core moment merge + (a, lo, hi) coefs for group g.
            Returns (first, last) instructions for scheduling-order pins."""
            ve = nc.gpsimd if on_pool else nc.vector
            S = len(GROUPS[g])
            F = S * 2
            gath = gathers[g]
            t4 = tmp.tile([P, 4, F], F32, tag=f"t4_{g}", name=f"t4_{g}")
            first = ve.tensor_tensor(t4[:], gath[:, 0:4], gath[:, 4:8],
                                     op=ALU.add)
            t2 = tmp.tile([P, 2, F], F32, tag=f"t2_{g}", name=f"t2_{g}")
            ve.tensor_tensor(t2[:], t4[:, 0:2], t4[:, 2:4], op=ALU.add)
            t1 = tmp.tile([P, F], F32, tag=f"t1_{g}", name=f"t1_{g}")
            ve.tensor_tensor(t1[:], t2[:, 0], t2[:, 1], op=ALU.add)
            gmean = gath[:].rearrange("p c (s f) -> p c s f", f=2)[:, :, :, 0]
            sq = tmp.tile([P, 8, S], F32, tag=f"sq_{g}", name=f"sq_{g}")
            ve.tensor_tensor(sq[:], gmean, gmean, op=ALU.mult)
            q4 = tmp.tile([P, 4, S], F32, tag=f"q4_{g}", name=f"q4_{g}")
            ve.tensor_tensor(q4[:], sq[:, 0:4], sq[:, 4:8], op=ALU.add)
            q2 = tmp.tile([P, 2, S], F32, tag=f"q2_{g}", name=f"q2_{g}")
            ve.tensor_tensor(q2[:], q4[:, 0:2], q4[:, 2:4], op=ALU.add)
            q1 = tmp.tile([P, S], F32, tag=f"q1_{g}", name=f"q1_{g}")
            ve.tensor_tensor(q1[:], q2[:, 0], q2[:, 1], op=ALU.add)

            t1v = t1[:].rearrange("p (s f) -> p s f", f=2)
            means = t1v[:, :, 0]
            vars_ = t1v[:, :, 1]
            mu = tmp.tile([P, S], F32, tag=f"mu{g}", name=f"mu{g}")
            ve.tensor_scalar(mu[:], means, 1.0 / n_cores, None, op0=ALU.mult)
            var = tmp.tile([P, S], F32, tag=f"var{g}", name=f"var{g}")
            ve.tensor_tensor(var[:], vars_, q1[:], op=ALU.add)
            ve.tensor_scalar(var[:], var[:], 1.0 / n_cores, None, op0=ALU.mult)
            musq = tmp.tile([P, S], F32, tag=f"musq{g}", name=f"musq{g}")
            ve.tensor_tensor(musq[:], mu[:], mu[:], op=ALU.mult)
            ve.tensor_tensor(var[:], var[:], musq[:], op=ALU.subtract)
            ve.tensor_scalar(var[:], var[:], EPS, None, op0=ALU.add)
            std = tmp.tile([P, S], F32, tag=f"std{g}", name=f"std{g}")
            nc.scalar.activation(std[:], var[:], AF.Sqrt)
            inv = tmp.tile([P, S], F32, tag=f"inv{g}", name=f"inv{g}")
            nc.vector.reciprocal(inv[:], std[:])   # DVE-only op
            off = GOFF[g]
            gam = gb_sb[:, off:off + S, 0]
            bet = gb_sb[:, off:off + S, 1]
            a_t = stat.tile([P, S], F32, tag=f"a{g}", name=f"a{g}")
            lo_t = stat.tile([P, S], F32, tag=f"lo{g}", name=f"lo{g}")
            hi_t = stat.tile([P, S], F32, tag=f"hi{g}", name=f"hi{g}")
            d_ = tmp.tile([P, S], F32, tag=f"d{g}", name=f"d{g}")
            ve.tensor_tensor(a_t[:], gam, inv[:], op=ALU.mult)
            ve.tensor_tensor(d_[:], mu[:], a_t[:], op=ALU.mult)
            ve.tensor_tensor(d_[:], bet, d_[:], op=ALU.subtract)
            inva = tmp.tile([P, S], F32, tag=f"inva{g}", name=f"inva{g}")
            nc.vector.reciprocal(inva[:], a_t[:])
            ve.tensor_scalar(lo_t[:], d_[:], -1.0, -1.0,
                             op0=ALU.mult, op1=ALU.add)
            ve.tensor_tensor(lo_t[:], lo_t[:], inva[:], op=ALU.mult)
            ve.tensor_scalar(hi_t[:], d_[:], -1.0, 1.0,
                             op0=ALU.mult, op1=ALU.add)
            last = ve.tensor_tensor(hi_t[:], hi_t[:], inva[:], op=ALU.mult)
            # s0 += per-mc sums of d_j (+ the pool branch's a*y_pool)
            if g == 0:
                dsum = tmp.tile([P, 2], F32, tag="dsum0", name="dsum0")
                dv = d_[:].rearrange("p (m s) -> p m s", m=2)
                ve.tensor_tensor(dsum[:], dv[:, :, 0], dv[:, :, 1], op=ALU.add)
                ve.tensor_tensor(dsum[:], dsum[:], dv[:, :, 2], op=ALU.add)
                last = ve.tensor_tensor(s0[:], s0[:], dsum[:], op=ALU.add)
                nc.vector.scalar_tensor_tensor(s0[:, 0:1], ypool[0][:],
                                               a_t[:, 0:1], s0[:, 0:1],
                                               op0=ALU.mult, op1=ALU.add)
                nc.vector.scalar_tensor_tensor(s0[:, 1:2], ypool[1][:],
                                               a_t[:, 3:4], s0[:, 1:2],
                                               op0=ALU.mult, op1=ALU.add)
            elif len(GROUPS[g]) == 2:
                last = ve.tensor_tensor(s0[:], s0[:], d_[:], op=ALU.add)
            else:
                mc = GROUPS[g][0][1]
                last = ve.tensor_tensor(s0[:, mc:mc + 1], s0[:, mc:mc + 1],
                                        d_[:, 0:1], op=ALU.add)
            coefs[g] = dict(a=a_t, lo=lo_t, hi=hi_t)
            return first, last

        NCH = 2
        CH = HW // NCH

        def apply_jobs(j, mcs=(0, 1)):
            """Chunked 3-pass apply for branch j as a list of closures:
            clip (ts), scale (ts) for r1 / scale+add-to-q for others."""
            jobs = []
            for mc in mcs:
                g, si = SLOT[(j, mc)]
                yt = y16_all[j][mc]
                for c in range(NCH):
                    def _clip(j=j, mc=mc, g=g, si=si, c=c, yt=yt):
                        nc.vector.tensor_scalar(
                            yt[:, c * CH:(c + 1) * CH],
                            yt[:, c * CH:(c + 1) * CH],
                            coefs[g]["lo"][:, si:si + 1],
                            coefs[g]["hi"][:, si:si + 1],
                            op0=ALU.max, op1=ALU.min)
                    jobs.append(_clip)
                for c in range(NCH):
                    def _scale(j=j, mc=mc, g=g, si=si, c=c, yt=yt):
                        nc.vector.tensor_scalar(
                            yt[:, c * CH:(c + 1) * CH],
                            yt[:, c * CH:(c + 1) * CH],
                            coefs[g]["a"][:, si:si + 1], None, op0=ALU.mult)
                    jobs.append(_scale)
                if j != 2:
                    for c in range(NCH):
                        def _add(j=j, mc=mc, c=c, yt=yt):
                            nc.vector.tensor_tensor(
                                q[mc][:, c * CH:(c + 1) * CH],
                                q[mc][:, c * CH:(c + 1) * CH],
                                yt[:, c * CH:(c + 1) * CH], op=ALU.add)
                        jobs.append(_add)
            return jobs

        def apply_tail(mc):
            """r12 half: sf = (a*clip(y) + s0) + q -> fp16 out + DMA."""
            g, si = SLOT[(5, mc)]
            c = coefs[g]
            yt = y16_all[5][mc]
            nc.vector.tensor_scalar(yt[:], yt[:], c["lo"][:, si:si + 1],
                                    c["hi"][:, si:si + 1],
                                    op0=ALU.max, op1=ALU.min)
            nc.vector.tensor_scalar(yt[:], yt[:], c["a"][:, si:si + 1],
                                    s0[:, mc:mc + 1],
                                    op0=ALU.mult, op1=ALU.add)
            for t in range(2):
                sf = sbout.tile([P, 2048], F16, tag="sf")
                nc.vector.tensor_tensor(sf[:], yt[:, t * 2048:(t + 1) * 2048],
                                        q[mc][:, t * 2048:(t + 1) * 2048],
                                        op=ALU.add)
                deng = nc.scalar if (mc, t) == (1, 1) else nc.sync
                deng.dma_start(
                    out.ap()[mc * P:(mc + 1) * P].rearrange(
                        "m h w -> m (h w)")[:, t * 2048:(t + 1) * 2048],
                    sf[:])

        def emit_section(units, side_jobs=()):
            side = list(side_jobs)
            for k, u in enumerate(units):
                emit_unit(*u)
                if side:
                    side.pop(0)()
            for fn in side:
                fn()

        def units_of(j, mcs=(0, 1)):
            return [(j, mc, t) for mc in mcs for t in range(NTILE)]

        # ---- emission ------------------------------------------------
        # S0: 1x1 + r1 interleaved (1x1 tile 0 first: needs only 12 rows)
        units_x = units_of(1)
        units_r1 = units_of(2)
        order = [units_x[0], units_x[NTILE]]
        rest_x = units_x[1:NTILE] + units_x[NTILE + 1:]
        for i, u in enumerate(units_r1):
            order.append(u)
            if i < len(rest_x):
                order.append(rest_x[i])
        emit_section(order)
        emit_pool_branch()
        issue_gather(0)

        # S1: r12 mc0 (its gather + coefs ride under S2/S3; its output
        # ships inside the final gather's window)
        emit_section(units_of(5, mcs=(0,)))
        issue_gather(1)

        # S2: r4.  merge-G0 on Pool once its gather lands; G0 applies
        # (r1 scale-in-place + 1x1 into q) chunk-interleave into S2/S3.
        emit_section(units_of(3, mcs=(0,))[:4])
        fetch_gather(0)
        m0 = emit_merge(0, on_pool=True)
        add_dep_helper(m0[0].ins, cc_insts[1].ins, sync=False, reason="pin after cc1")
        emit_section(units_of(3, mcs=(0,))[4:] + units_of(3, mcs=(1,)),
                     apply_jobs(2) + apply_jobs(1))
        issue_gather(2)
        add_dep_helper(cc_insts[2].ins, m0[1].ins, sync=False, reason="pin after m0")

        # S3: r8.  merge-G2 on Pool; r4 applies interleave into S3; the
        # r12-mc0 coef merge (G1) rides late in S3 on DVE.
        emit_section(units_of(4, mcs=(0,))[:4])
        fetch_gather(2)
        m2 = emit_merge(2, on_pool=True)
        add_dep_helper(m2[0].ins, cc_insts[2].ins, sync=False, reason="pin after cc2")
        emit_section(units_of(4, mcs=(0,))[4:] + units_of(4, mcs=(1,))[:4],
                     apply_jobs(3))
        fetch_gather(1, eng=nc.sync)
        emit_merge(1, on_pool=False)
        emit_section(units_of(4, mcs=(1,))[4:])
        issue_gather(3)
        add_dep_helper(cc_insts[3].ins, m2[1].ins, sync=False, reason="pin after m2")

        # S4: r12 mc1.  merge-G3 (r8) on DVE mid-section; r8 applies
        # chunk-interleave; then the mc0 tail output ships.
        emit_section(units_of(5, mcs=(1,))[:4])
        fetch_gather(3)
        emit_merge(3, on_pool=False)
        emit_section(units_of(5, mcs=(1,))[4:], apply_jobs(4))
        issue_gather(4)

        apply_tail(0)                    # mc0 output inside G4's window
        fetch_gather(4)
        emit_merge(4, on_pool=False)
        apply_tail(1)

    nc.compile()
    return nc


def pack_weights(w_pool, w1, w3):
    """Host filter transform: sign -> DoubleRow k-interleave, fp8.

    wt[k, t*2+mc, i, m] = sign(W_t[mc*128+m, i*128+k]); block (t*2+mc) is
    the stationary [2, 128] operand for logical tap t / out-chunk mc.
    """
    mats = [np.sign(np.asarray(w_pool, np.float32).reshape(COUT, CIN)),
            np.sign(np.asarray(w1, np.float32).reshape(COUT, CIN))]
    w3 = np.asarray(w3, np.float32)
    for i in range(len(RATES)):
        for ky in range(3):
            for kx in range(3):
                mats.append(np.sign(w3[i, :, :, ky, kx]))
    wt = np.zeros((P, NT * 2, 2, P), np.float32)  # [k, blk, i, m]
    for t, m in enumerate(mats):
        for mc in range(2):
            for i in range(2):
                blk = m[mc * P:(mc + 1) * P, i * P:(i + 1) * P]   # [m, k]
                wt[:, t * 2 + mc, i, :] = blk.T
    return wt.astype(mybir.dt.np(FP8))


def pack_gb(g_pool, b_pool, g1, b1, g3, b3):
    """gamma/beta packed [P, slot, 2] in global gather-slot order."""
    gs = [g_pool, g1] + [g3[i] for i in range(len(RATES))]
    bs = [b_pool, b1] + [b3[i] for i in range(len(RATES))]
    slots = [jm for members in GROUPS for jm in members]
    gb = np.zeros((P, 12, 2), np.float32)
    for s, (j, mc) in enumerate(slots):
        gb[:, s, 0] = np.asarray(gs[j], np.float32)[mc * P:(mc + 1) * P]
        gb[:, s, 1] = np.asarray(bs[j], np.float32)[mc * P:(mc + 1) * P]
    return gb


_NC = None


def _get_nc():
    global _NC
    if _NC is None:
        _NC = build(N_CORES)
    return _NC


def make_in_maps(x, w_pool, g_pool, b_pool, w1, g1, b1, w3, g3, b3):
    x = np.asarray(x, np.float32)
    wt = pack_weights(w_pool, w1, w3)
    gb = pack_gb(g_pool, b_pool, g1, b1, g3, b3)
    return [
        {"xs": np.ascontiguousarray(x[c]), "wt": wt, "gb": gb}
        for c in range(x.shape[0])
    ]


def kernel(x, w_pool, g_pool, b_pool, w1, g1, b1, w3, g3, b3):
    nc = _get_nc()
    in_maps = make_in_maps(x, w_pool, g_pool, b_pool, w1, g1, b1, w3, g3, b3)
    res = run_bass_kernel_spmd(nc, in_maps, core_ids=list(range(N_CORES)))
    return np.stack([res.results[c]["out"] for c in range(N_CORES)],
                    axis=0).astype(np.float32)
